# revision 26
# baseline (speedup 1.0000x reference)
"""C3DLoss kernel for Trainium2 — 8-core batch-parallel, raw-Bass implementation.

Per core = one batch frame b (tgt pairing partner tb = b^1):
    partial = sum over both terms (same-frame, cross-frame), all 25 shifts
              delta in [-2,2]^2, all pixels p of
        mref(p) * mq(p+delta) * exp(-50*(|xyz_r(p)-xyz_q(p+d)|^2
                                         + |rgb_r(p)-rgb_q(p+d)|^2))
    loss = -(sum of partials) / max(sum(depth_gt_mask), 1)

v8 design notes (HW-measured behavior; ~1.05ms vs the 2.70ms baseline):
  - DVE runs every fp16 op at 2 elem/cycle/lane (~633ns per 1216-col op)
    with ~150ns fixed overhead; engines contend on the memory system, so
    work stays off GpSimd entirely (its Q7 cores poison SBUF bandwidth)
    and the DVE stream is batched: the 5 dx-shifts of one (term, dy)
    group are computed in ONE instruction (stride-0 broadcast ref outer
    dim, stride-1 shifted query outer dim).
  - Masks folded into feature channel 3 (ra3=+20*(1-mref),
    qa3=-20*(1-mq)); compact 2-D sub layouts (no halo columns).
  - Per 5-slot group: DVE does subB5, subA5 and (for 8 of 10 groups per
    slab) sqA5; ScalarE squares the rgb diff group in one Square
    activation, squares the A-diffs of the other 2 groups (ACT_GS), and
    runs exp+accumulate in-place on PSUM per (batch, chunk) unit.  This
    balances DVE at ~1.01ms against ScalarE at ~0.99ms.
  - PE per 4-slot batch: all selA matmuls then all selB (weight reuse),
    contiguous rhs chunks, 8 PSUM banks, then_inc completion signaling.
"""

import sys

for _p in ("/opt/trn_rl_repo", "/opt/pypackages"):
    if _p not in sys.path:
        sys.path.insert(0, _p)

from contextlib import ExitStack

import numpy as np

import concourse.bass as bass
import concourse.mybir as mybir
from concourse.ap import AP
from concourse.alu_op_type import AluOpType

F32 = mybir.dt.float32
F16 = mybir.dt.float16

R = 2
G = 32            # W-blocks; one shift-slot = 32 PSUM partitions
CA = 4            # tile A channels: x, y, z, mask
CB = 3            # tile B channels: r, g, b
SBATCH = 4        # shift slots per 128-partition PSUM bank
QUINT = 5         # dx-shifts batched per DVE/Act instruction
NPSUM = 8         # rotating PSUM banks (unit = (batch, chunk))
NG = 3            # rotation depth for the sqa group tiles
NDB = 4           # rotation depth for the db/sqb group tiles
ACT_GS = (4, 9)   # per-slab group indices whose sqA runs on ScalarE
MK = 20.0         # mask channel scale; (2*MK)^2 = 1600 >> 1/50
EXP_SCALE = -50.0


class Cfg:
    def __init__(self, H=352, W=1216, HS=32):
        assert W % G == 0 and H % HS == 0
        self.H, self.W, self.HS = H, W, HS
        self.WB = W // G                      # 38
        self.WBH = self.WB + 2 * R            # 42
        self.Hp = H + 2 * R                   # 356
        self.NSLAB = H // HS                  # 11
        self.NQ = G * self.Hp * self.WBH      # haloed plane elems
        self.QF = (HS + 2 * R) * self.WBH     # query tile free size 1512
        self.RF = HS * self.WBH               # ref tile free size 1344
        self.SF = HS * self.WB                # compact slot size 1216
        cw = (512 // self.WB) * self.WB       # 494
        self.chunks = []
        o = 0
        while o < self.SF:
            self.chunks.append((o, min(cw, self.SF - o)))
            o += cw
        self.NC = len(self.chunks)            # 3
        self.slots = [(t, dy, dx) for t in (0, 1)
                      for dy in range(-R, R + 1) for dx in range(-R, R + 1)]
        self.NS = len(self.slots)             # 50
        assert self.NS % QUINT == 0
        self.NGS = self.NS // QUINT           # 10 groups per slab
        self.batches = [self.slots[i:i + SBATCH]
                        for i in range(0, self.NS, SBATCH)]
        self.NB = len(self.batches)           # 13
        self.TOTS = self.NSLAB * self.NS      # 550 slots
        self.TOTB = self.NSLAB * self.NB      # 143 batches
        self.TOTG = self.NSLAB * self.NGS     # 110 groups
        self.n_acc = self.TOTB * self.NC      # 429 acc columns
        # per-group sqA producer: ScalarE for ACT_GS, DVE otherwise
        self.g_act = [(g % self.NGS) in ACT_GS for g in range(self.TOTG)]
        self.cnt_va = []   # cumulative Act-sqA count after group g
        self.cnt_vq = []   # cumulative DVE-sqA count after group g
        ca = cq = 0
        for g in range(self.TOTG):
            if self.g_act[g]:
                ca += 1
            else:
                cq += 1
            self.cnt_va.append(ca)
            self.cnt_vq.append(cq)

    def slot_batch(self, J):
        return (J // self.NS) * self.NB + (J % self.NS) // SBATCH


def _apv(t_ap, p0, pcnt, free_dims, free_off=0):
    pstride = t_ap.ap[0][0]
    base = t_ap.offset + p0 * pstride + free_off
    return AP(t_ap.tensor, base, [[pstride, pcnt]] + [list(d) for d in free_dims])


def _dram_ap(handle, offset, dims):
    a = handle[:]
    return AP(a.tensor, a.offset + offset, [list(d) for d in dims])


def make_selA():
    s = np.zeros((CA * G, G), dtype=np.float16)
    for c in range(CA):
        for g in range(G):
            s[c * G + g, g] = 1
    return s


def make_selB():
    s = np.zeros((CB * G, G), dtype=np.float16)
    for c in range(CB):
        for g in range(G):
            s[c * G + g, g] = 1
    return s


def emit(nc: bass.Bass, cfg: Cfg):
    HS, WB, WBH, Hp = cfg.HS, cfg.WB, cfg.WBH, cfg.Hp
    NQ, QF, RF, SF = cfg.NQ, cfg.QF, cfg.RF, cfg.SF
    NSLAB, NB, NC, NS, NGS = cfg.NSLAB, cfg.NB, cfg.NC, cfg.NS, cfg.NGS
    Act = mybir.ActivationFunctionType
    HpW = Hp * WBH
    Q5 = QUINT * SF

    dp = nc.declare_dram_parameter
    qa_d = dp("qa_d", [2, CA, NQ], F16, isOutput=False)
    ra_d = dp("ra_d", [2, CA, NQ], F16, isOutput=False)
    qb_d = dp("qb_d", [CB, NQ], F16, isOutput=False)
    rbt_d = dp("rbt_d", [CB, NQ], F16, isOutput=False)
    selA_d = dp("selA_d", [CA * G, G], F16, isOutput=False)
    selB_d = dp("selB_d", [CB * G, G], F16, isOutput=False)
    out_d = dp("out_d", [128, 1], F32, isOutput=True)

    LD = 6
    NCONST = 2

    with ExitStack() as ex:
        E = ex.enter_context
        qa_s = [[E(nc.sbuf_tensor(f"qa{t}{p}", [CA * G, QF], F16))
                 for p in range(2)] for t in range(2)]
        ra_s = [[E(nc.sbuf_tensor(f"ra{t}{p}", [CA * G, RF], F16))
                 for p in range(2)] for t in range(2)]
        qb_s = [E(nc.sbuf_tensor(f"qb{p}", [CB * G, QF], F16))
                for p in range(2)]
        rbt_s = [E(nc.sbuf_tensor(f"rbt{p}", [CB * G, RF], F16))
                 for p in range(2)]
        da_s = E(nc.sbuf_tensor("da", [CA * G, Q5], F16))
        dact_s = [E(nc.sbuf_tensor(f"dact{i}", [CA * G, Q5], F16))
                  for i in range(2)]
        db_s = [E(nc.sbuf_tensor(f"db{i}", [CB * G, Q5], F16))
                for i in range(NDB)]
        sqa_s = [E(nc.sbuf_tensor(f"sqa{i}", [CA * G, Q5], F16))
                 for i in range(NG)]
        sqb_s = [E(nc.sbuf_tensor(f"sqb{i}", [CB * G, Q5], F16))
                 for i in range(NDB)]
        acc_s = E(nc.sbuf_tensor("acc", [128, cfg.n_acc], F32))
        res_s = E(nc.sbuf_tensor("res", [128, 1], F32))
        selA_s = E(nc.sbuf_tensor("selA", [CA * G, G], F16))
        selB_s = E(nc.sbuf_tensor("selB", [CB * G, G], F16))
        ps_s = [E(nc.psum_tensor(f"ps{i}", [128, 512], F32))
                for i in range(NPSUM)]

        sL = E(nc.semaphore("sL"))
        sLC = E(nc.semaphore("sLC"))
        sL0 = E(nc.semaphore("sL0"))
        sL1 = E(nc.semaphore("sL1"))
        sG = E(nc.semaphore("sG"))
        sV = E(nc.semaphore("sV"))    # DVE subB5 done (1/group)
        sVq = E(nc.semaphore("sVq"))  # DVE sqA5 done (1/DVE-sq group) + final
        sVda = E(nc.semaphore("sVda"))  # DVE subA5 done (1/Act-sq group)
        sVa = E(nc.semaphore("sVa"))  # Act sqA group done (1/Act-sq group)
        sAq = E(nc.semaphore("sAq"))  # Act sqB group done (1/group)
        sP = E(nc.semaphore("sP"))    # PE batch done (1/batch)
        sA = E(nc.semaphore("sA"))    # Act exp units done (1/unit)
        blk = E(nc.Block())

        # access-pattern builders ------------------------------------------
        def q5_ap(tile, pcnt, dy):
            # 5 dx-shifted query windows (dx=-2..2) as one 3-D pattern
            off = (R + dy) * WBH
            return _apv(tile.ap(), 0, pcnt,
                        [[1, QUINT], [WBH, HS], [1, WB]], off)

        def r5_ap(tile, pcnt, off=R):
            # ref window broadcast across the 5 dx shifts
            return _apv(tile.ap(), 0, pcnt,
                        [[0, QUINT], [WBH, HS], [1, WB]], off)

        def d5_out(tile, pcnt):
            return _apv(tile.ap(), 0, pcnt, [[SF, QUINT], [WB, HS], [1, WB]])

        def stream(tile, pcnt, n, off=0):
            return _apv(tile.ap(), 0, pcnt, [[1, n]], off)

        def rgbref5_ap(t, ph):
            if t == 0:
                # t=0 ref rgb == query rgb plane at center
                return _apv(qb_s[ph].ap(), 0, CB * G,
                            [[0, QUINT], [WBH, HS], [1, WB]], R * WBH + R)
            return r5_ap(rbt_s[ph], CB * G)

        @blk.gpsimd
        def _(gp):
            gp.memset(acc_s.ap(), 0.0)
            gp.memset(res_s.ap(), 0.0)
            gp.drain()
            gp.sem_inc(sG, 1)

        @blk.sync
        def _(sp):
            sp.dma_start(selA_s[:], selA_d[:]).then_inc(sLC, 16)
            sp.dma_start(selB_s[:], selB_d[:]).then_inc(sLC, 16)
            for s in range(NSLAB):
                ph = s % 2
                if s >= 2:
                    # PE progress implies DVE is done reading slab s-2 tiles
                    sp.wait_ge(sP, NB * (s - 1))
                r0 = s * HS
                sLs = sL0 if ph == 0 else sL1
                for t in range(2):
                    sp.dma_start(
                        qa_s[t][ph].ap(),
                        _dram_ap(qa_d, t * CA * NQ + r0 * WBH,
                                 [[NQ, CA], [HpW, G], [1, QF]])
                    ).then_inc(sLs, 16)
                    sp.dma_start(
                        ra_s[t][ph].ap(),
                        _dram_ap(ra_d, t * CA * NQ + (r0 + R) * WBH,
                                 [[NQ, CA], [HpW, G], [1, RF]])
                    ).then_inc(sLs, 16)
                sp.dma_start(
                    qb_s[ph].ap(),
                    _dram_ap(qb_d, r0 * WBH, [[NQ, CB], [HpW, G], [1, QF]])
                ).then_inc(sLs, 16)
                sp.dma_start(
                    rbt_s[ph].ap(),
                    _dram_ap(rbt_d, (r0 + R) * WBH,
                             [[NQ, CB], [HpW, G], [1, RF]])
                ).then_inc(sLs, 16)
            sp.wait_ge(sVq, cfg.cnt_vq[-1] + 1)
            sp.dma_start(out_d[:], res_s.ap()).then_inc(sL, 16)

        @blk.vector
        def _(ve):
            for s in range(NSLAB):
                ph = s % 2
                sLs = sL0 if ph == 0 else sL1
                ve.wait_ge(sLs, 16 * LD * (s // 2 + 1))
                for gs in range(NGS):
                    g5 = s * NGS + gs
                    t, dy, _ = cfg.slots[gs * QUINT]
                    if g5 >= NDB:
                        # db tile recycling: Act done squaring group g5-NDB
                        ve.wait_ge(sAq, g5 - NDB + 1)
                    if g5 >= NG and not cfg.g_act[g5]:
                        # sqa tile recycling: PE done with group g5-NG
                        ve.wait_ge(sP, cfg.slot_batch(QUINT * (g5 - NG + 1) - 1) + 1)
                    nc.vector.tensor_tensor(
                        d5_out(db_s[g5 % NDB], CB * G),
                        rgbref5_ap(t, ph),
                        q5_ap(qb_s[ph], CB * G, dy),
                        AluOpType.subtract).then_inc(sV, 1)
                    if cfg.g_act[g5]:
                        ia = cfg.cnt_va[g5] - 1
                        if ia - 2 >= 0:
                            ve.wait_ge(sVa, ia - 1)
                        nc.vector.tensor_tensor(
                            d5_out(dact_s[ia % 2], CA * G),
                            r5_ap(ra_s[t][ph], CA * G),
                            q5_ap(qa_s[t][ph], CA * G, dy),
                            AluOpType.subtract).then_inc(sVda, 1)
                    else:
                        nc.vector.tensor_tensor(
                            d5_out(da_s, CA * G),
                            r5_ap(ra_s[t][ph], CA * G),
                            q5_ap(qa_s[t][ph], CA * G, dy),
                            AluOpType.subtract)
                        nc.vector.tensor_mul(
                            stream(sqa_s[g5 % NG], CA * G, Q5),
                            stream(da_s, CA * G, Q5),
                            stream(da_s, CA * G, Q5)).then_inc(sVq, 1)
            ve.wait_ge(sA, cfg.TOTB * NC)
            nc.vector.tensor_reduce(
                res_s.ap(), acc_s.ap(), axis=mybir.AxisListType.X,
                op=AluOpType.add).then_inc(sVq, 1)

        @blk.tensor
        def _(pe):
            pe.wait_ge(sLC, 16 * NCONST)
            last_vq = last_va = last_aq = 0
            for s in range(NSLAB):
                for b in range(NB):
                    gb = s * NB + b
                    L = len(cfg.batches[b])
                    gJ0 = s * NS + b * SBATCH
                    if NC * gb - NPSUM + NC >= 1:
                        pe.wait_ge(sA, NC * gb - NPSUM + NC)
                    for jj in range(L):
                        J = gJ0 + jj
                        g5 = J // QUINT
                        if cfg.g_act[g5]:
                            need = cfg.cnt_va[g5]
                            if need > last_va:
                                pe.wait_ge(sVa, need)
                                last_va = need
                        else:
                            need = cfg.cnt_vq[g5]
                            if need > last_vq:
                                pe.wait_ge(sVq, need)
                                last_vq = need
                        for c, (co, cn) in enumerate(cfg.chunks):
                            u = gb * NC + c
                            nc.tensor.matmul(
                                ps_s[u % NPSUM][G * jj:G * (jj + 1), :cn],
                                selA_s[:],
                                stream(sqa_s[(J // QUINT) % NG], CA * G, cn,
                                       (J % QUINT) * SF + co),
                                start=True, stop=False, skip_group_check=True,
                                tile_position=(0, G * jj))
                    for jj in range(L):
                        J = gJ0 + jj
                        need = J // QUINT + 1
                        if need > last_aq:
                            pe.wait_ge(sAq, need)
                            last_aq = need
                        for c, (co, cn) in enumerate(cfg.chunks):
                            u = gb * NC + c
                            mm = nc.tensor.matmul(
                                ps_s[u % NPSUM][G * jj:G * (jj + 1), :cn],
                                selB_s[:],
                                stream(sqb_s[(J // QUINT) % NDB], CB * G, cn,
                                       (J % QUINT) * SF + co),
                                start=False, stop=True, skip_group_check=True,
                                tile_position=(0, G * jj))
                            if jj == L - 1 and c == NC - 1:
                                mm.then_inc(sP, 1)

        @blk.scalar
        def _(ac):
            ac.wait_ge(sG, 1)
            for s in range(NSLAB):
                gi = 0
                for b in range(NB):
                    gb = s * NB + b
                    L = len(cfg.batches[b])
                    gJ0 = s * NS + b * SBATCH
                    need_g = min(((gJ0 + L - 1) % NS) // QUINT + 1, NGS - 1)
                    if b == NB - 1:
                        need_g = NGS - 1
                    while gi <= need_g:
                        g5 = s * NGS + gi
                        ac.wait_ge(sV, g5 + 1)
                        if g5 >= NDB:
                            # sqb tile recycling: PE done with group g5-NDB
                            ac.wait_ge(
                                sP,
                                cfg.slot_batch(QUINT * (g5 - NDB + 1) - 1) + 1)
                        nc.scalar.activation(
                            stream(sqb_s[g5 % NDB], CB * G, Q5),
                            stream(db_s[g5 % NDB], CB * G, Q5),
                            Act.Square).then_inc(sAq, 1)
                        if cfg.g_act[g5]:
                            ia = cfg.cnt_va[g5] - 1
                            ac.wait_ge(sVda, ia + 1)
                            if g5 >= NG:
                                # sqa tile recycling for the Act-written group
                                ac.wait_ge(
                                    sP,
                                    cfg.slot_batch(QUINT * (g5 - NG + 1) - 1)
                                    + 1)
                            nc.scalar.activation(
                                stream(sqa_s[g5 % NG], CA * G, Q5),
                                stream(dact_s[ia % 2], CA * G, Q5),
                                Act.Square).then_inc(sVa, 1)
                        gi += 1
                    pb = G * L
                    ac.wait_ge(sP, gb + 1)
                    for c, (co, cn) in enumerate(cfg.chunks):
                        u = gb * NC + c
                        nc.scalar.activation(
                            ps_s[u % NPSUM][:pb, :cn],
                            ps_s[u % NPSUM][:pb, :cn],
                            Act.Exp, scale=EXP_SCALE,
                            accum_out=acc_s[:pb, u:u + 1]).then_inc(sA, 1)
    return nc


# ---------------- host side ----------------

def _block_q(plane, cfg):
    """[H, W] -> flat blocked+haloed [G*Hp*WBH] fp16, zero-padded borders."""
    p = np.zeros((cfg.Hp, cfg.W + 2 * R), dtype=np.float32)
    p[R:R + cfg.H, R:R + cfg.W] = plane
    out = np.empty((G, cfg.Hp, cfg.WBH), dtype=np.float16)
    for g in range(G):
        out[g] = p[:, g * cfg.WB:g * cfg.WB + cfg.WBH]
    return np.ascontiguousarray(out).reshape(-1)


def host_precompute(rgb, depth, depth_gt, depth_mask, depth_gt_mask,
                    xy1_grid, Ts, cfg, b):
    tb = b ^ 1
    xy1 = np.asarray(xy1_grid[b], np.float32)
    dep = np.asarray(depth[b, 0], np.float32)
    dgt_b = np.asarray(depth_gt[b, 0], np.float32)
    dgt_t = np.asarray(depth_gt[tb, 0], np.float32)
    mp = np.asarray(depth_mask[b, 0], np.float32)
    mg_b = np.asarray(depth_gt_mask[b, 0], np.float32)
    mg_t = np.asarray(depth_gt_mask[tb, 0], np.float32)

    xyz_p = xy1 * dep
    T21 = (np.linalg.inv(np.asarray(Ts[tb], np.float64)) @
           np.asarray(Ts[b], np.float64)).astype(np.float32)
    Rm, tv = T21[:3, :3], T21[:3, 3]
    txyz = np.einsum('ij,jhw->ihw', Rm, xyz_p).astype(np.float32) \
        + tv[:, None, None].astype(np.float32)
    pos = (txyz[2] > 0).astype(np.float32) * mp

    qa = np.empty((2, CA, cfg.NQ), np.float16)
    ra = np.empty((2, CA, cfg.NQ), np.float16)
    for c in range(3):
        qa[0, c] = _block_q(xyz_p[c], cfg)
        qa[1, c] = _block_q(txyz[c], cfg)
        ra[0, c] = _block_q(xy1[c] * dgt_b, cfg)
        ra[1, c] = _block_q(xy1[c] * dgt_t, cfg)
    # mask channel: (ra3 - qa3)^2 = 0 iff both masks pass, else >= 400
    qa[0, 3] = -MK * (1.0 - _block_q(mp, cfg))
    qa[1, 3] = -MK * (1.0 - _block_q(pos, cfg))
    ra[0, 3] = MK * (1.0 - _block_q(mg_b, cfg))
    ra[1, 3] = MK * (1.0 - _block_q(mg_t, cfg))
    qb = np.stack([_block_q(np.asarray(rgb[b, c], np.float32), cfg)
                   for c in range(3)])
    rbt = np.stack([_block_q(np.asarray(rgb[tb, c], np.float32), cfg)
                    for c in range(3)])
    return {"qa_d": qa, "ra_d": ra, "qb_d": qb, "rbt_d": rbt,
            "selA_d": make_selA(), "selB_d": make_selB()}


def make_in_maps(rgb, depth, depth_gt, depth_mask, depth_gt_mask, xy1_grid, Ts,
                 cfg, n_cores=8):
    return [host_precompute(rgb, depth, depth_gt, depth_mask, depth_gt_mask,
                            xy1_grid, Ts, cfg, b) for b in range(n_cores)]


_CACHED = {}


def _get_nc(cfg_key=(352, 1216, 32)):
    if cfg_key not in _CACHED:
        cfg = Cfg(*cfg_key)
        nc = bass.Bass()
        emit(nc, cfg)
        _CACHED[cfg_key] = (nc, cfg)
    return _CACHED[cfg_key]


def kernel(rgb, depth, depth_gt, depth_mask, depth_gt_mask, xy1_grid, Ts,
           **run_kwargs):
    from concourse.bass_utils import run_bass_kernel_spmd
    nc, cfg = _get_nc()
    maps = make_in_maps(rgb, depth, depth_gt, depth_mask, depth_gt_mask,
                        xy1_grid, Ts, cfg)
    res = run_bass_kernel_spmd(nc, maps, list(range(8)), **run_kwargs)
    total = np.float64(0.0)
    for r in res.results:
        total += np.float64(r["out_d"][:, 0].sum())
    n_gt = max(np.asarray(depth_gt_mask, np.float64).sum(), 1.0)
    loss = -total / n_gt
    kernel.last_results = res
    return np.float32(loss)


# revision 27
# speedup vs baseline: 1.0005x; 1.0005x over previous
"""C3DLoss kernel for Trainium2 — 8-core batch-parallel, raw-Bass implementation.

Per core = one batch frame b (tgt pairing partner tb = b^1):
    partial = sum over both terms (same-frame, cross-frame), all 25 shifts
              delta in [-2,2]^2, all pixels p of
        mref(p) * mq(p+delta) * exp(-50*(|xyz_r(p)-xyz_q(p+d)|^2
                                         + |rgb_r(p)-rgb_q(p+d)|^2))
    loss = -(sum of partials) / max(sum(depth_gt_mask), 1)

v8 design notes (HW-measured behavior; ~1.05ms vs the 2.70ms baseline):
  - DVE runs every fp16 op at 2 elem/cycle/lane (~633ns per 1216-col op)
    with ~150ns fixed overhead; engines contend on the memory system, so
    work stays off GpSimd entirely (its Q7 cores poison SBUF bandwidth)
    and the DVE stream is batched: the 5 dx-shifts of one (term, dy)
    group are computed in ONE instruction (stride-0 broadcast ref outer
    dim, stride-1 shifted query outer dim).
  - Masks folded into feature channel 3 (ra3=+20*(1-mref),
    qa3=-20*(1-mq)); compact 2-D sub layouts (no halo columns).
  - Per 5-slot group: DVE does subB5, subA5 and (for 8 of 10 groups per
    slab) sqA5; ScalarE squares the rgb diff group in one Square
    activation, squares the A-diffs of the other 2 groups (ACT_GS), and
    runs exp+accumulate in-place on PSUM per (batch, chunk) unit.  This
    balances DVE at ~1.01ms against ScalarE at ~0.99ms.
  - PE per 4-slot batch: all selA matmuls then all selB (weight reuse),
    contiguous rhs chunks, 8 PSUM banks, then_inc completion signaling.
"""

import sys

for _p in ("/opt/trn_rl_repo", "/opt/pypackages"):
    if _p not in sys.path:
        sys.path.insert(0, _p)

from contextlib import ExitStack

import numpy as np

import concourse.bass as bass
import concourse.mybir as mybir
from concourse.ap import AP
from concourse.alu_op_type import AluOpType

F32 = mybir.dt.float32
F16 = mybir.dt.float16

R = 2
G = 32            # W-blocks; one shift-slot = 32 PSUM partitions
CA = 4            # tile A channels: x, y, z, mask
CB = 3            # tile B channels: r, g, b
SBATCH = 4        # shift slots per 128-partition PSUM bank
QUINT = 5         # dx-shifts batched per DVE/Act instruction
NPSUM = 8         # rotating PSUM banks (unit = (batch, chunk))
NG = 3            # rotation depth for the per-group work tiles
ACT_GS = (4, 9)   # per-slab group indices whose sqA runs on ScalarE
MK = 20.0         # mask channel scale; (2*MK)^2 = 1600 >> 1/50
EXP_SCALE = -50.0


class Cfg:
    def __init__(self, H=352, W=1216, HS=32):
        assert W % G == 0 and H % HS == 0
        self.H, self.W, self.HS = H, W, HS
        self.WB = W // G                      # 38
        self.WBH = self.WB + 2 * R            # 42
        self.Hp = H + 2 * R                   # 356
        self.NSLAB = H // HS                  # 11
        self.NQ = G * self.Hp * self.WBH      # haloed plane elems
        self.QF = (HS + 2 * R) * self.WBH     # query tile free size 1512
        self.RF = HS * self.WBH               # ref tile free size 1344
        self.SF = HS * self.WB                # compact slot size 1216
        cw = (512 // self.WB) * self.WB       # 494
        self.chunks = []
        o = 0
        while o < self.SF:
            self.chunks.append((o, min(cw, self.SF - o)))
            o += cw
        self.NC = len(self.chunks)            # 3
        self.slots = [(t, dy, dx) for t in (0, 1)
                      for dy in range(-R, R + 1) for dx in range(-R, R + 1)]
        self.NS = len(self.slots)             # 50
        assert self.NS % QUINT == 0
        self.NGS = self.NS // QUINT           # 10 groups per slab
        self.batches = [self.slots[i:i + SBATCH]
                        for i in range(0, self.NS, SBATCH)]
        self.NB = len(self.batches)           # 13
        self.TOTS = self.NSLAB * self.NS      # 550 slots
        self.TOTB = self.NSLAB * self.NB      # 143 batches
        self.TOTG = self.NSLAB * self.NGS     # 110 groups
        self.n_acc = self.TOTB * self.NC      # 429 acc columns
        # per-group sqA producer: ScalarE for ACT_GS, DVE otherwise
        self.g_act = [(g % self.NGS) in ACT_GS for g in range(self.TOTG)]
        self.cnt_va = []   # cumulative Act-sqA count after group g
        self.cnt_vq = []   # cumulative DVE-sqA count after group g
        ca = cq = 0
        for g in range(self.TOTG):
            if self.g_act[g]:
                ca += 1
            else:
                cq += 1
            self.cnt_va.append(ca)
            self.cnt_vq.append(cq)

    def slot_batch(self, J):
        return (J // self.NS) * self.NB + (J % self.NS) // SBATCH


def _apv(t_ap, p0, pcnt, free_dims, free_off=0):
    pstride = t_ap.ap[0][0]
    base = t_ap.offset + p0 * pstride + free_off
    return AP(t_ap.tensor, base, [[pstride, pcnt]] + [list(d) for d in free_dims])


def _dram_ap(handle, offset, dims):
    a = handle[:]
    return AP(a.tensor, a.offset + offset, [list(d) for d in dims])


def make_selA():
    s = np.zeros((CA * G, G), dtype=np.float16)
    for c in range(CA):
        for g in range(G):
            s[c * G + g, g] = 1
    return s


def make_selB():
    s = np.zeros((CB * G, G), dtype=np.float16)
    for c in range(CB):
        for g in range(G):
            s[c * G + g, g] = 1
    return s


def emit(nc: bass.Bass, cfg: Cfg):
    HS, WB, WBH, Hp = cfg.HS, cfg.WB, cfg.WBH, cfg.Hp
    NQ, QF, RF, SF = cfg.NQ, cfg.QF, cfg.RF, cfg.SF
    NSLAB, NB, NC, NS, NGS = cfg.NSLAB, cfg.NB, cfg.NC, cfg.NS, cfg.NGS
    Act = mybir.ActivationFunctionType
    HpW = Hp * WBH
    Q5 = QUINT * SF

    dp = nc.declare_dram_parameter
    qa_d = dp("qa_d", [2, CA, NQ], F16, isOutput=False)
    ra_d = dp("ra_d", [2, CA, NQ], F16, isOutput=False)
    qb_d = dp("qb_d", [CB, NQ], F16, isOutput=False)
    rbt_d = dp("rbt_d", [CB, NQ], F16, isOutput=False)
    selA_d = dp("selA_d", [CA * G, G], F16, isOutput=False)
    selB_d = dp("selB_d", [CB * G, G], F16, isOutput=False)
    out_d = dp("out_d", [128, 1], F32, isOutput=True)

    LD = 6
    NCONST = 2

    with ExitStack() as ex:
        E = ex.enter_context
        qa_s = [[E(nc.sbuf_tensor(f"qa{t}{p}", [CA * G, QF], F16))
                 for p in range(2)] for t in range(2)]
        ra_s = [[E(nc.sbuf_tensor(f"ra{t}{p}", [CA * G, RF], F16))
                 for p in range(2)] for t in range(2)]
        qb_s = [E(nc.sbuf_tensor(f"qb{p}", [CB * G, QF], F16))
                for p in range(2)]
        rbt_s = [E(nc.sbuf_tensor(f"rbt{p}", [CB * G, RF], F16))
                 for p in range(2)]
        da_s = E(nc.sbuf_tensor("da", [CA * G, Q5], F16))
        dact_s = [E(nc.sbuf_tensor(f"dact{i}", [CA * G, Q5], F16))
                  for i in range(2)]
        db_s = [E(nc.sbuf_tensor(f"db{i}", [CB * G, Q5], F16))
                for i in range(NG)]
        sqa_s = [E(nc.sbuf_tensor(f"sqa{i}", [CA * G, Q5], F16))
                 for i in range(NG)]
        sqb_s = [E(nc.sbuf_tensor(f"sqb{i}", [CB * G, Q5], F16))
                 for i in range(NG)]
        kt_s = [E(nc.sbuf_tensor(f"kt{i}", [128, 512], F16)) for i in range(2)]
        acc_s = E(nc.sbuf_tensor("acc", [128, cfg.n_acc], F32))
        res_s = E(nc.sbuf_tensor("res", [128, 1], F32))
        selA_s = E(nc.sbuf_tensor("selA", [CA * G, G], F16))
        selB_s = E(nc.sbuf_tensor("selB", [CB * G, G], F16))
        ps_s = [E(nc.psum_tensor(f"ps{i}", [128, 512], F32))
                for i in range(NPSUM)]

        sL = E(nc.semaphore("sL"))
        sLC = E(nc.semaphore("sLC"))
        sL0 = E(nc.semaphore("sL0"))
        sL1 = E(nc.semaphore("sL1"))
        sG = E(nc.semaphore("sG"))
        sV = E(nc.semaphore("sV"))    # DVE subB5 done (1/group)
        sVq = E(nc.semaphore("sVq"))  # DVE sqA5 done (1/DVE-sq group) + final
        sVda = E(nc.semaphore("sVda"))  # DVE subA5 done (1/Act-sq group)
        sVa = E(nc.semaphore("sVa"))  # Act sqA group done (1/Act-sq group)
        sAq = E(nc.semaphore("sAq"))  # Act sqB group done (1/group)
        sP = E(nc.semaphore("sP"))    # PE batch done (1/batch)
        sA = E(nc.semaphore("sA"))    # Act exp units done (1/unit)
        blk = E(nc.Block())

        # access-pattern builders ------------------------------------------
        def q5_ap(tile, pcnt, dy):
            # 5 dx-shifted query windows (dx=-2..2) as one 3-D pattern
            off = (R + dy) * WBH
            return _apv(tile.ap(), 0, pcnt,
                        [[1, QUINT], [WBH, HS], [1, WB]], off)

        def r5_ap(tile, pcnt, off=R):
            # ref window broadcast across the 5 dx shifts
            return _apv(tile.ap(), 0, pcnt,
                        [[0, QUINT], [WBH, HS], [1, WB]], off)

        def d5_out(tile, pcnt):
            return _apv(tile.ap(), 0, pcnt, [[SF, QUINT], [WB, HS], [1, WB]])

        def stream(tile, pcnt, n, off=0):
            return _apv(tile.ap(), 0, pcnt, [[1, n]], off)

        def rgbref5_ap(t, ph):
            if t == 0:
                # t=0 ref rgb == query rgb plane at center
                return _apv(qb_s[ph].ap(), 0, CB * G,
                            [[0, QUINT], [WBH, HS], [1, WB]], R * WBH + R)
            return r5_ap(rbt_s[ph], CB * G)

        @blk.gpsimd
        def _(gp):
            gp.memset(acc_s.ap(), 0.0)
            gp.memset(res_s.ap(), 0.0)
            gp.drain()
            gp.sem_inc(sG, 1)

        @blk.sync
        def _(sp):
            sp.dma_start(selA_s[:], selA_d[:]).then_inc(sLC, 16)
            sp.dma_start(selB_s[:], selB_d[:]).then_inc(sLC, 16)
            for s in range(NSLAB):
                ph = s % 2
                if s >= 2:
                    # PE progress implies DVE is done reading slab s-2 tiles
                    sp.wait_ge(sP, NB * (s - 1))
                r0 = s * HS
                sLs = sL0 if ph == 0 else sL1
                for t in range(2):
                    sp.dma_start(
                        qa_s[t][ph].ap(),
                        _dram_ap(qa_d, t * CA * NQ + r0 * WBH,
                                 [[NQ, CA], [HpW, G], [1, QF]])
                    ).then_inc(sLs, 16)
                    sp.dma_start(
                        ra_s[t][ph].ap(),
                        _dram_ap(ra_d, t * CA * NQ + (r0 + R) * WBH,
                                 [[NQ, CA], [HpW, G], [1, RF]])
                    ).then_inc(sLs, 16)
                sp.dma_start(
                    qb_s[ph].ap(),
                    _dram_ap(qb_d, r0 * WBH, [[NQ, CB], [HpW, G], [1, QF]])
                ).then_inc(sLs, 16)
                sp.dma_start(
                    rbt_s[ph].ap(),
                    _dram_ap(rbt_d, (r0 + R) * WBH,
                             [[NQ, CB], [HpW, G], [1, RF]])
                ).then_inc(sLs, 16)
            sp.wait_ge(sVq, cfg.cnt_vq[-1] + 1)
            sp.dma_start(out_d[:], res_s.ap()).then_inc(sL, 16)

        @blk.vector
        def _(ve):
            for s in range(NSLAB):
                ph = s % 2
                sLs = sL0 if ph == 0 else sL1
                ve.wait_ge(sLs, 16 * LD * (s // 2 + 1))
                for gs in range(NGS):
                    g5 = s * NGS + gs
                    t, dy, _ = cfg.slots[gs * QUINT]
                    if g5 >= NG:
                        # sqa/db tile recycling: PE / Act done with g5-NG
                        ve.wait_ge(sP, cfg.slot_batch(QUINT * (g5 - NG + 1) - 1) + 1)
                        ve.wait_ge(sAq, g5 - NG + 1)
                    nc.vector.tensor_tensor(
                        d5_out(db_s[g5 % NG], CB * G),
                        rgbref5_ap(t, ph),
                        q5_ap(qb_s[ph], CB * G, dy),
                        AluOpType.subtract).then_inc(sV, 1)
                    if cfg.g_act[g5]:
                        ia = cfg.cnt_va[g5] - 1
                        if ia - 2 >= 0:
                            ve.wait_ge(sVa, ia - 1)
                        nc.vector.tensor_tensor(
                            d5_out(dact_s[ia % 2], CA * G),
                            r5_ap(ra_s[t][ph], CA * G),
                            q5_ap(qa_s[t][ph], CA * G, dy),
                            AluOpType.subtract).then_inc(sVda, 1)
                    else:
                        nc.vector.tensor_tensor(
                            d5_out(da_s, CA * G),
                            r5_ap(ra_s[t][ph], CA * G),
                            q5_ap(qa_s[t][ph], CA * G, dy),
                            AluOpType.subtract)
                        nc.vector.tensor_mul(
                            stream(sqa_s[g5 % NG], CA * G, Q5),
                            stream(da_s, CA * G, Q5),
                            stream(da_s, CA * G, Q5)).then_inc(sVq, 1)
            ve.wait_ge(sA, cfg.TOTB * NC)
            nc.vector.tensor_reduce(
                res_s.ap(), acc_s.ap(), axis=mybir.AxisListType.X,
                op=AluOpType.add).then_inc(sVq, 1)

        @blk.tensor
        def _(pe):
            pe.wait_ge(sLC, 16 * NCONST)
            last_vq = last_va = last_aq = 0
            for s in range(NSLAB):
                for b in range(NB):
                    gb = s * NB + b
                    L = len(cfg.batches[b])
                    gJ0 = s * NS + b * SBATCH
                    if NC * gb - NPSUM + NC >= 1:
                        pe.wait_ge(sA, NC * gb - NPSUM + NC)
                    for jj in range(L):
                        J = gJ0 + jj
                        g5 = J // QUINT
                        if cfg.g_act[g5]:
                            need = cfg.cnt_va[g5]
                            if need > last_va:
                                pe.wait_ge(sVa, need)
                                last_va = need
                        else:
                            need = cfg.cnt_vq[g5]
                            if need > last_vq:
                                pe.wait_ge(sVq, need)
                                last_vq = need
                        for c, (co, cn) in enumerate(cfg.chunks):
                            u = gb * NC + c
                            nc.tensor.matmul(
                                ps_s[u % NPSUM][G * jj:G * (jj + 1), :cn],
                                selA_s[:],
                                stream(sqa_s[(J // QUINT) % NG], CA * G, cn,
                                       (J % QUINT) * SF + co),
                                start=True, stop=False, skip_group_check=True,
                                tile_position=(0, G * jj))
                    for jj in range(L):
                        J = gJ0 + jj
                        need = J // QUINT + 1
                        if need > last_aq:
                            pe.wait_ge(sAq, need)
                            last_aq = need
                        for c, (co, cn) in enumerate(cfg.chunks):
                            u = gb * NC + c
                            mm = nc.tensor.matmul(
                                ps_s[u % NPSUM][G * jj:G * (jj + 1), :cn],
                                selB_s[:],
                                stream(sqb_s[(J // QUINT) % NG], CB * G, cn,
                                       (J % QUINT) * SF + co),
                                start=False, stop=True, skip_group_check=True,
                                tile_position=(0, G * jj))
                            if jj == L - 1 and c == NC - 1:
                                mm.then_inc(sP, 1)

        @blk.scalar
        def _(ac):
            ac.wait_ge(sG, 1)
            for s in range(NSLAB):
                gi = 0
                for b in range(NB):
                    gb = s * NB + b
                    L = len(cfg.batches[b])
                    gJ0 = s * NS + b * SBATCH
                    need_g = min(((gJ0 + L - 1) % NS) // QUINT + 1, NGS - 1)
                    if b == NB - 1:
                        need_g = NGS - 1
                    while gi <= need_g:
                        g5 = s * NGS + gi
                        ac.wait_ge(sV, g5 + 1)
                        if g5 >= NG:
                            ac.wait_ge(
                                sP,
                                cfg.slot_batch(QUINT * (g5 - NG + 1) - 1) + 1)
                        nc.scalar.activation(
                            stream(sqb_s[g5 % NG], CB * G, Q5),
                            stream(db_s[g5 % NG], CB * G, Q5),
                            Act.Square).then_inc(sAq, 1)
                        if cfg.g_act[g5]:
                            ia = cfg.cnt_va[g5] - 1
                            ac.wait_ge(sVda, ia + 1)
                            nc.scalar.activation(
                                stream(sqa_s[g5 % NG], CA * G, Q5),
                                stream(dact_s[ia % 2], CA * G, Q5),
                                Act.Square).then_inc(sVa, 1)
                        gi += 1
                    pb = G * L
                    ac.wait_ge(sP, gb + 1)
                    for c, (co, cn) in enumerate(cfg.chunks):
                        u = gb * NC + c
                        nc.scalar.activation(
                            ps_s[u % NPSUM][:pb, :cn],
                            ps_s[u % NPSUM][:pb, :cn],
                            Act.Exp, scale=EXP_SCALE,
                            accum_out=acc_s[:pb, u:u + 1]).then_inc(sA, 1)
    return nc


# ---------------- host side ----------------

def _block_q(plane, cfg):
    """[H, W] -> flat blocked+haloed [G*Hp*WBH] fp16, zero-padded borders."""
    p = np.zeros((cfg.Hp, cfg.W + 2 * R), dtype=np.float32)
    p[R:R + cfg.H, R:R + cfg.W] = plane
    out = np.empty((G, cfg.Hp, cfg.WBH), dtype=np.float16)
    for g in range(G):
        out[g] = p[:, g * cfg.WB:g * cfg.WB + cfg.WBH]
    return np.ascontiguousarray(out).reshape(-1)


def host_precompute(rgb, depth, depth_gt, depth_mask, depth_gt_mask,
                    xy1_grid, Ts, cfg, b):
    tb = b ^ 1
    xy1 = np.asarray(xy1_grid[b], np.float32)
    dep = np.asarray(depth[b, 0], np.float32)
    dgt_b = np.asarray(depth_gt[b, 0], np.float32)
    dgt_t = np.asarray(depth_gt[tb, 0], np.float32)
    mp = np.asarray(depth_mask[b, 0], np.float32)
    mg_b = np.asarray(depth_gt_mask[b, 0], np.float32)
    mg_t = np.asarray(depth_gt_mask[tb, 0], np.float32)

    xyz_p = xy1 * dep
    T21 = (np.linalg.inv(np.asarray(Ts[tb], np.float64)) @
           np.asarray(Ts[b], np.float64)).astype(np.float32)
    Rm, tv = T21[:3, :3], T21[:3, 3]
    txyz = np.einsum('ij,jhw->ihw', Rm, xyz_p).astype(np.float32) \
        + tv[:, None, None].astype(np.float32)
    pos = (txyz[2] > 0).astype(np.float32) * mp

    qa = np.empty((2, CA, cfg.NQ), np.float16)
    ra = np.empty((2, CA, cfg.NQ), np.float16)
    for c in range(3):
        qa[0, c] = _block_q(xyz_p[c], cfg)
        qa[1, c] = _block_q(txyz[c], cfg)
        ra[0, c] = _block_q(xy1[c] * dgt_b, cfg)
        ra[1, c] = _block_q(xy1[c] * dgt_t, cfg)
    # mask channel: (ra3 - qa3)^2 = 0 iff both masks pass, else >= 400
    qa[0, 3] = -MK * (1.0 - _block_q(mp, cfg))
    qa[1, 3] = -MK * (1.0 - _block_q(pos, cfg))
    ra[0, 3] = MK * (1.0 - _block_q(mg_b, cfg))
    ra[1, 3] = MK * (1.0 - _block_q(mg_t, cfg))
    qb = np.stack([_block_q(np.asarray(rgb[b, c], np.float32), cfg)
                   for c in range(3)])
    rbt = np.stack([_block_q(np.asarray(rgb[tb, c], np.float32), cfg)
                    for c in range(3)])
    return {"qa_d": qa, "ra_d": ra, "qb_d": qb, "rbt_d": rbt,
            "selA_d": make_selA(), "selB_d": make_selB()}


def make_in_maps(rgb, depth, depth_gt, depth_mask, depth_gt_mask, xy1_grid, Ts,
                 cfg, n_cores=8):
    return [host_precompute(rgb, depth, depth_gt, depth_mask, depth_gt_mask,
                            xy1_grid, Ts, cfg, b) for b in range(n_cores)]


_CACHED = {}


def _get_nc(cfg_key=(352, 1216, 32)):
    if cfg_key not in _CACHED:
        cfg = Cfg(*cfg_key)
        nc = bass.Bass()
        emit(nc, cfg)
        _CACHED[cfg_key] = (nc, cfg)
    return _CACHED[cfg_key]


def kernel(rgb, depth, depth_gt, depth_mask, depth_gt_mask, xy1_grid, Ts,
           **run_kwargs):
    from concourse.bass_utils import run_bass_kernel_spmd
    nc, cfg = _get_nc()
    maps = make_in_maps(rgb, depth, depth_gt, depth_mask, depth_gt_mask,
                        xy1_grid, Ts, cfg)
    res = run_bass_kernel_spmd(nc, maps, list(range(8)), **run_kwargs)
    total = np.float64(0.0)
    for r in res.results:
        total += np.float64(r["out_d"][:, 0].sum())
    n_gt = max(np.asarray(depth_gt_mask, np.float64).sum(), 1.0)
    loss = -total / n_gt
    kernel.last_results = res
    return np.float32(loss)


# revision 29
# speedup vs baseline: 1.0163x; 1.0158x over previous
"""C3DLoss kernel for Trainium2 — 8-core batch-parallel, raw-Bass implementation.

Per core = one batch frame b (tgt pairing partner tb = b^1):
    partial = sum over both terms (same-frame, cross-frame), all 25 shifts
              delta in [-2,2]^2, all pixels p of
        mref(p) * mq(p+delta) * exp(-50*(|xyz_r(p)-xyz_q(p+d)|^2
                                         + |rgb_r(p)-rgb_q(p+d)|^2))
    loss = -(sum of partials) / max(sum(depth_gt_mask), 1)

v8 design notes (HW-measured behavior; ~1.05ms vs the 2.70ms baseline):
  - DVE runs every fp16 op at 2 elem/cycle/lane (~633ns per 1216-col op)
    with ~150ns fixed overhead; engines contend on the memory system, so
    work stays off GpSimd entirely (its Q7 cores poison SBUF bandwidth)
    and the DVE stream is batched: the 5 dx-shifts of one (term, dy)
    group are computed in ONE instruction (stride-0 broadcast ref outer
    dim, stride-1 shifted query outer dim).
  - Masks folded into feature channel 3 (ra3=+20*(1-mref),
    qa3=-20*(1-mq)); compact 2-D sub layouts (no halo columns).
  - Per 5-slot group: DVE does subB5, subA5 and (for 8 of 10 groups per
    slab) sqA5; ScalarE squares the rgb diff group in one Square
    activation, squares the A-diffs of the other 2 groups (ACT_GS), and
    runs exp+accumulate in-place on PSUM per (batch, chunk) unit.  This
    balances DVE at ~1.01ms against ScalarE at ~0.99ms.
  - PE per 4-slot batch: all selA matmuls then all selB (weight reuse),
    contiguous rhs chunks, 8 PSUM banks, then_inc completion signaling.
"""

import sys

for _p in ("/opt/trn_rl_repo", "/opt/pypackages"):
    if _p not in sys.path:
        sys.path.insert(0, _p)

from contextlib import ExitStack

import numpy as np

import concourse.bass as bass
import concourse.mybir as mybir
from concourse.ap import AP
from concourse.alu_op_type import AluOpType

F32 = mybir.dt.float32
F16 = mybir.dt.float16

R = 2
G = 32            # W-blocks; one shift-slot = 32 PSUM partitions
CA = 4            # tile A channels: x, y, z, mask
CB = 3            # tile B channels: r, g, b
SBATCH = 4        # shift slots per 128-partition PSUM bank
QUINT = 5         # dx-shifts batched per DVE/Act instruction
NPSUM = 7         # rotating PSUM banks (unit = (batch, chunk))
NG = 3            # rotation depth for the per-group work tiles
ACT_GS = (2, 5, 8)  # per-slab group indices whose sqA runs on ScalarE
MK = 20.0         # mask channel scale; (2*MK)^2 = 1600 >> 1/50
EXP_SCALE = -50.0


class Cfg:
    def __init__(self, H=352, W=1216, HS=32):
        assert W % G == 0 and H % HS == 0
        self.H, self.W, self.HS = H, W, HS
        self.WB = W // G                      # 38
        self.WBH = self.WB + 2 * R            # 42
        self.Hp = H + 2 * R                   # 356
        self.NSLAB = H // HS                  # 11
        self.NQ = G * self.Hp * self.WBH      # haloed plane elems
        self.QF = (HS + 2 * R) * self.WBH     # query tile free size 1512
        self.RF = HS * self.WBH               # ref tile free size 1344
        self.SF = HS * self.WB                # compact slot size 1216
        cw = (512 // self.WB) * self.WB       # 494
        self.chunks = []
        o = 0
        while o < self.SF:
            self.chunks.append((o, min(cw, self.SF - o)))
            o += cw
        self.NC = len(self.chunks)            # 3
        self.slots = [(t, dy, dx) for t in (0, 1)
                      for dy in range(-R, R + 1) for dx in range(-R, R + 1)]
        self.NS = len(self.slots)             # 50
        assert self.NS % QUINT == 0
        self.NGS = self.NS // QUINT           # 10 groups per slab
        self.batches = [self.slots[i:i + SBATCH]
                        for i in range(0, self.NS, SBATCH)]
        self.NB = len(self.batches)           # 13
        self.TOTS = self.NSLAB * self.NS      # 550 slots
        self.TOTB = self.NSLAB * self.NB      # 143 batches
        self.TOTG = self.NSLAB * self.NGS     # 110 groups
        self.n_acc = self.TOTB * self.NC      # 429 acc columns
        # per-group sqA producer: ScalarE for ACT_GS, DVE otherwise
        self.g_act = [(g % self.NGS) in ACT_GS for g in range(self.TOTG)]
        self.cnt_va = []   # cumulative Act-sqA count after group g
        self.cnt_vq = []   # cumulative DVE-sqA count after group g
        ca = cq = 0
        for g in range(self.TOTG):
            if self.g_act[g]:
                ca += 1
            else:
                cq += 1
            self.cnt_va.append(ca)
            self.cnt_vq.append(cq)

    def slot_batch(self, J):
        return (J // self.NS) * self.NB + (J % self.NS) // SBATCH


def _apv(t_ap, p0, pcnt, free_dims, free_off=0):
    pstride = t_ap.ap[0][0]
    base = t_ap.offset + p0 * pstride + free_off
    return AP(t_ap.tensor, base, [[pstride, pcnt]] + [list(d) for d in free_dims])


def _dram_ap(handle, offset, dims):
    a = handle[:]
    return AP(a.tensor, a.offset + offset, [list(d) for d in dims])


def make_selA():
    s = np.zeros((CA * G, G), dtype=np.float16)
    for c in range(CA):
        for g in range(G):
            s[c * G + g, g] = 1
    return s


def make_selB():
    s = np.zeros((CB * G, G), dtype=np.float16)
    for c in range(CB):
        for g in range(G):
            s[c * G + g, g] = 1
    return s


def emit(nc: bass.Bass, cfg: Cfg):
    HS, WB, WBH, Hp = cfg.HS, cfg.WB, cfg.WBH, cfg.Hp
    NQ, QF, RF, SF = cfg.NQ, cfg.QF, cfg.RF, cfg.SF
    NSLAB, NB, NC, NS, NGS = cfg.NSLAB, cfg.NB, cfg.NC, cfg.NS, cfg.NGS
    Act = mybir.ActivationFunctionType
    HpW = Hp * WBH
    Q5 = QUINT * SF

    dp = nc.declare_dram_parameter
    qa_d = dp("qa_d", [2, CA, NQ], F16, isOutput=False)
    ra_d = dp("ra_d", [2, CA, NQ], F16, isOutput=False)
    qb_d = dp("qb_d", [CB, NQ], F16, isOutput=False)
    rbt_d = dp("rbt_d", [CB, NQ], F16, isOutput=False)
    selA_d = dp("selA_d", [CA * G, G], F16, isOutput=False)
    selB_d = dp("selB_d", [CB * G, G], F16, isOutput=False)
    ones_d = dp("ones_d", [128, 1], F16, isOutput=False)
    out_d = dp("out_d", [128, 1], F32, isOutput=True)

    LD = 6
    NCONST = 3

    with ExitStack() as ex:
        E = ex.enter_context
        qa_s = [[E(nc.sbuf_tensor(f"qa{t}{p}", [CA * G, QF], F16))
                 for p in range(2)] for t in range(2)]
        ra_s = [[E(nc.sbuf_tensor(f"ra{t}{p}", [CA * G, RF], F16))
                 for p in range(2)] for t in range(2)]
        qb_s = [E(nc.sbuf_tensor(f"qb{p}", [CB * G, QF], F16))
                for p in range(2)]
        rbt_s = [E(nc.sbuf_tensor(f"rbt{p}", [CB * G, RF], F16))
                 for p in range(2)]
        da_s = E(nc.sbuf_tensor("da", [CA * G, Q5], F16))
        dact_s = [E(nc.sbuf_tensor(f"dact{i}", [CA * G, Q5], F16))
                  for i in range(2)]
        db_s = [E(nc.sbuf_tensor(f"db{i}", [CB * G, Q5], F16))
                for i in range(NG)]
        sqa_s = [E(nc.sbuf_tensor(f"sqa{i}", [CA * G, Q5], F16))
                 for i in range(NG)]
        sqb_s = [E(nc.sbuf_tensor(f"sqb{i}", [CB * G, Q5], F16))
                 for i in range(NG)]
        kt_s = [E(nc.sbuf_tensor(f"kt{i}", [128, 512], F16)) for i in range(4)]
        acc_s = E(nc.sbuf_tensor("acc", [128, cfg.n_acc], F32))
        res_s = E(nc.sbuf_tensor("res", [128, 1], F32))
        selA_s = E(nc.sbuf_tensor("selA", [CA * G, G], F16))
        selB_s = E(nc.sbuf_tensor("selB", [CB * G, G], F16))
        ones_s = E(nc.sbuf_tensor("ones", [128, 1], F16))
        ps_s = [E(nc.psum_tensor(f"ps{i}", [128, 512], F32))
                for i in range(NPSUM)]
        acc_ps = E(nc.psum_tensor("accps", [128, 512], F32))

        sL = E(nc.semaphore("sL"))
        sLC = E(nc.semaphore("sLC"))
        sL0 = E(nc.semaphore("sL0"))
        sL1 = E(nc.semaphore("sL1"))
        sG = E(nc.semaphore("sG"))
        sV = E(nc.semaphore("sV"))    # DVE subB5 done (1/group)
        sVq = E(nc.semaphore("sVq"))  # DVE sqA5 done (1/DVE-sq group) + final
        sVda = E(nc.semaphore("sVda"))  # DVE subA5 done (1/Act-sq group)
        sVa = E(nc.semaphore("sVa"))  # Act sqA group done (1/Act-sq group)
        sAq = E(nc.semaphore("sAq"))  # Act sqB group done (1/group)
        sP = E(nc.semaphore("sP"))    # PE batch done (1/batch)
        sA = E(nc.semaphore("sA"))    # Act exp units done (1/unit)
        sKm = E(nc.semaphore("sKm"))  # PE kt-sum matmuls done (1/unit)
        blk = E(nc.Block())

        # access-pattern builders ------------------------------------------
        def q5_ap(tile, pcnt, dy):
            # 5 dx-shifted query windows (dx=-2..2) as one 3-D pattern
            off = (R + dy) * WBH
            return _apv(tile.ap(), 0, pcnt,
                        [[1, QUINT], [WBH, HS], [1, WB]], off)

        def r5_ap(tile, pcnt, off=R):
            # ref window broadcast across the 5 dx shifts
            return _apv(tile.ap(), 0, pcnt,
                        [[0, QUINT], [WBH, HS], [1, WB]], off)

        def d5_out(tile, pcnt):
            return _apv(tile.ap(), 0, pcnt, [[SF, QUINT], [WB, HS], [1, WB]])

        def stream(tile, pcnt, n, off=0):
            return _apv(tile.ap(), 0, pcnt, [[1, n]], off)

        def rgbref5_ap(t, ph):
            if t == 0:
                # t=0 ref rgb == query rgb plane at center
                return _apv(qb_s[ph].ap(), 0, CB * G,
                            [[0, QUINT], [WBH, HS], [1, WB]], R * WBH + R)
            return r5_ap(rbt_s[ph], CB * G)

        @blk.gpsimd
        def _(gp):
            gp.memset(acc_s.ap(), 0.0)
            gp.memset(res_s.ap(), 0.0)
            gp.drain()
            gp.sem_inc(sG, 1)

        @blk.sync
        def _(sp):
            sp.dma_start(selA_s[:], selA_d[:]).then_inc(sLC, 16)
            sp.dma_start(selB_s[:], selB_d[:]).then_inc(sLC, 16)
            sp.dma_start(ones_s[:], ones_d[:]).then_inc(sLC, 16)
            for s in range(NSLAB):
                ph = s % 2
                if s >= 2:
                    # PE progress implies DVE is done reading slab s-2 tiles
                    sp.wait_ge(sP, NB * (s - 1))
                r0 = s * HS
                sLs = sL0 if ph == 0 else sL1
                for t in range(2):
                    sp.dma_start(
                        qa_s[t][ph].ap(),
                        _dram_ap(qa_d, t * CA * NQ + r0 * WBH,
                                 [[NQ, CA], [HpW, G], [1, QF]])
                    ).then_inc(sLs, 16)
                    sp.dma_start(
                        ra_s[t][ph].ap(),
                        _dram_ap(ra_d, t * CA * NQ + (r0 + R) * WBH,
                                 [[NQ, CA], [HpW, G], [1, RF]])
                    ).then_inc(sLs, 16)
                sp.dma_start(
                    qb_s[ph].ap(),
                    _dram_ap(qb_d, r0 * WBH, [[NQ, CB], [HpW, G], [1, QF]])
                ).then_inc(sLs, 16)
                sp.dma_start(
                    rbt_s[ph].ap(),
                    _dram_ap(rbt_d, (r0 + R) * WBH,
                             [[NQ, CB], [HpW, G], [1, RF]])
                ).then_inc(sLs, 16)
            sp.wait_ge(sVq, cfg.cnt_vq[-1] + 1)
            sp.dma_start(out_d[:], res_s.ap()).then_inc(sL, 16)

        @blk.vector
        def _(ve):
            for s in range(NSLAB):
                ph = s % 2
                sLs = sL0 if ph == 0 else sL1
                ve.wait_ge(sLs, 16 * LD * (s // 2 + 1))
                for gs in range(NGS):
                    g5 = s * NGS + gs
                    t, dy, _ = cfg.slots[gs * QUINT]
                    if g5 >= NG:
                        # sqa/db tile recycling: PE / Act done with g5-NG
                        ve.wait_ge(sP, cfg.slot_batch(QUINT * (g5 - NG + 1) - 1) + 1)
                        ve.wait_ge(sAq, g5 - NG + 1)
                    nc.vector.tensor_tensor(
                        d5_out(db_s[g5 % NG], CB * G),
                        rgbref5_ap(t, ph),
                        q5_ap(qb_s[ph], CB * G, dy),
                        AluOpType.subtract).then_inc(sV, 1)
                    if cfg.g_act[g5]:
                        ia = cfg.cnt_va[g5] - 1
                        if ia - 2 >= 0:
                            ve.wait_ge(sVa, ia - 1)
                        nc.vector.tensor_tensor(
                            d5_out(dact_s[ia % 2], CA * G),
                            r5_ap(ra_s[t][ph], CA * G),
                            q5_ap(qa_s[t][ph], CA * G, dy),
                            AluOpType.subtract).then_inc(sVda, 1)
                    else:
                        nc.vector.tensor_tensor(
                            d5_out(da_s, CA * G),
                            r5_ap(ra_s[t][ph], CA * G),
                            q5_ap(qa_s[t][ph], CA * G, dy),
                            AluOpType.subtract)
                        nc.vector.tensor_mul(
                            stream(sqa_s[g5 % NG], CA * G, Q5),
                            stream(da_s, CA * G, Q5),
                            stream(da_s, CA * G, Q5)).then_inc(sVq, 1)
            ve.wait_ge(sKm, cfg.TOTB * NC)
            nc.vector.tensor_reduce(
                res_s[0:1, :], acc_ps[0:1, :494], axis=mybir.AxisListType.X,
                op=AluOpType.add).then_inc(sVq, 1)

        @blk.tensor
        def _(pe):
            pe.wait_ge(sLC, 16 * NCONST)
            last_vq = last_va = last_aq = 0
            for s in range(NSLAB):
                for b in range(NB):
                    gb = s * NB + b
                    L = len(cfg.batches[b])
                    gJ0 = s * NS + b * SBATCH
                    if NC * gb - NPSUM + NC >= 1:
                        pe.wait_ge(sA, NC * gb - NPSUM + NC)
                    for jj in range(L):
                        J = gJ0 + jj
                        g5 = J // QUINT
                        if cfg.g_act[g5]:
                            need = cfg.cnt_va[g5]
                            if need > last_va:
                                pe.wait_ge(sVa, need)
                                last_va = need
                        else:
                            need = cfg.cnt_vq[g5]
                            if need > last_vq:
                                pe.wait_ge(sVq, need)
                                last_vq = need
                        for c, (co, cn) in enumerate(cfg.chunks):
                            u = gb * NC + c
                            nc.tensor.matmul(
                                ps_s[u % NPSUM][G * jj:G * (jj + 1), :cn],
                                selA_s[:],
                                stream(sqa_s[(J // QUINT) % NG], CA * G, cn,
                                       (J % QUINT) * SF + co),
                                start=True, stop=False, skip_group_check=True,
                                tile_position=(0, G * jj))
                    for jj in range(L):
                        J = gJ0 + jj
                        need = J // QUINT + 1
                        if need > last_aq:
                            pe.wait_ge(sAq, need)
                            last_aq = need
                        for c, (co, cn) in enumerate(cfg.chunks):
                            u = gb * NC + c
                            mm = nc.tensor.matmul(
                                ps_s[u % NPSUM][G * jj:G * (jj + 1), :cn],
                                selB_s[:],
                                stream(sqb_s[(J // QUINT) % NG], CB * G, cn,
                                       (J % QUINT) * SF + co),
                                start=False, stop=True, skip_group_check=True,
                                tile_position=(0, G * jj))
                            if jj == L - 1 and c == NC - 1:
                                mm.then_inc(sP, 1)
                    if gb >= 1:
                        pe.wait_ge(sA, NC * gb)
                        pbp = G * len(cfg.batches[(gb - 1) % NB])
                        for c, (co, cn) in enumerate(cfg.chunks):
                            u = (gb - 1) * NC + c
                            nc.tensor.matmul(
                                acc_ps[0:1, :cn], ones_s[:pbp, :],
                                _apv(kt_s[u % 4].ap(), 0, pbp, [[1, cn]]),
                                start=(u == 0), stop=(u == cfg.n_acc - 1),
                                skip_group_check=True,
                                tile_position=(0, 0)).then_inc(sKm, 1)

            gbl = cfg.TOTB - 1
            pe.wait_ge(sA, NC * gbl + NC)
            pbl = G * len(cfg.batches[NB - 1])
            for c, (co, cn) in enumerate(cfg.chunks):
                u = gbl * NC + c
                nc.tensor.matmul(
                    acc_ps[0:1, :cn], ones_s[:pbl, :],
                    _apv(kt_s[u % 4].ap(), 0, pbl, [[1, cn]]),
                    start=(u == 0), stop=(u == cfg.n_acc - 1),
                    skip_group_check=True,
                    tile_position=(0, 0)).then_inc(sKm, 1)

        @blk.scalar
        def _(ac):
            ac.wait_ge(sG, 1)
            for s in range(NSLAB):
                gi = 0
                for b in range(NB):
                    gb = s * NB + b
                    L = len(cfg.batches[b])
                    gJ0 = s * NS + b * SBATCH
                    need_g = min(((gJ0 + L - 1) % NS) // QUINT + 1, NGS - 1)
                    if b == NB - 1:
                        need_g = NGS - 1
                    while gi <= need_g:
                        g5 = s * NGS + gi
                        ac.wait_ge(sV, g5 + 1)
                        if g5 >= NG:
                            ac.wait_ge(
                                sP,
                                cfg.slot_batch(QUINT * (g5 - NG + 1) - 1) + 1)
                        nc.scalar.activation(
                            stream(sqb_s[g5 % NG], CB * G, Q5),
                            stream(db_s[g5 % NG], CB * G, Q5),
                            Act.Square).then_inc(sAq, 1)
                        if cfg.g_act[g5]:
                            ia = cfg.cnt_va[g5] - 1
                            ac.wait_ge(sVda, ia + 1)
                            nc.scalar.activation(
                                stream(sqa_s[g5 % NG], CA * G, Q5),
                                stream(dact_s[ia % 2], CA * G, Q5),
                                Act.Square).then_inc(sVa, 1)
                        gi += 1
                    pb = G * L
                    ac.wait_ge(sP, gb + 1)
                    for c, (co, cn) in enumerate(cfg.chunks):
                        u = gb * NC + c
                        if u - 4 >= 0:
                            ac.wait_ge(sKm, u - 3)
                        nc.scalar.activation(
                            kt_s[u % 4][:pb, :cn],
                            ps_s[u % NPSUM][:pb, :cn],
                            Act.Exp, scale=EXP_SCALE).then_inc(sA, 1)
    return nc


# ---------------- host side ----------------

def _block_q(plane, cfg):
    """[H, W] -> flat blocked+haloed [G*Hp*WBH] fp16, zero-padded borders."""
    p = np.zeros((cfg.Hp, cfg.W + 2 * R), dtype=np.float32)
    p[R:R + cfg.H, R:R + cfg.W] = plane
    out = np.empty((G, cfg.Hp, cfg.WBH), dtype=np.float16)
    for g in range(G):
        out[g] = p[:, g * cfg.WB:g * cfg.WB + cfg.WBH]
    return np.ascontiguousarray(out).reshape(-1)


def host_precompute(rgb, depth, depth_gt, depth_mask, depth_gt_mask,
                    xy1_grid, Ts, cfg, b):
    tb = b ^ 1
    xy1 = np.asarray(xy1_grid[b], np.float32)
    dep = np.asarray(depth[b, 0], np.float32)
    dgt_b = np.asarray(depth_gt[b, 0], np.float32)
    dgt_t = np.asarray(depth_gt[tb, 0], np.float32)
    mp = np.asarray(depth_mask[b, 0], np.float32)
    mg_b = np.asarray(depth_gt_mask[b, 0], np.float32)
    mg_t = np.asarray(depth_gt_mask[tb, 0], np.float32)

    xyz_p = xy1 * dep
    T21 = (np.linalg.inv(np.asarray(Ts[tb], np.float64)) @
           np.asarray(Ts[b], np.float64)).astype(np.float32)
    Rm, tv = T21[:3, :3], T21[:3, 3]
    txyz = np.einsum('ij,jhw->ihw', Rm, xyz_p).astype(np.float32) \
        + tv[:, None, None].astype(np.float32)
    pos = (txyz[2] > 0).astype(np.float32) * mp

    qa = np.empty((2, CA, cfg.NQ), np.float16)
    ra = np.empty((2, CA, cfg.NQ), np.float16)
    for c in range(3):
        qa[0, c] = _block_q(xyz_p[c], cfg)
        qa[1, c] = _block_q(txyz[c], cfg)
        ra[0, c] = _block_q(xy1[c] * dgt_b, cfg)
        ra[1, c] = _block_q(xy1[c] * dgt_t, cfg)
    # mask channel: (ra3 - qa3)^2 = 0 iff both masks pass, else >= 400
    qa[0, 3] = -MK * (1.0 - _block_q(mp, cfg))
    qa[1, 3] = -MK * (1.0 - _block_q(pos, cfg))
    ra[0, 3] = MK * (1.0 - _block_q(mg_b, cfg))
    ra[1, 3] = MK * (1.0 - _block_q(mg_t, cfg))
    qb = np.stack([_block_q(np.asarray(rgb[b, c], np.float32), cfg)
                   for c in range(3)])
    rbt = np.stack([_block_q(np.asarray(rgb[tb, c], np.float32), cfg)
                    for c in range(3)])
    return {"qa_d": qa, "ra_d": ra, "qb_d": qb, "rbt_d": rbt,
            "selA_d": make_selA(), "selB_d": make_selB(),
            "ones_d": np.ones((128, 1), np.float16)}


def make_in_maps(rgb, depth, depth_gt, depth_mask, depth_gt_mask, xy1_grid, Ts,
                 cfg, n_cores=8):
    return [host_precompute(rgb, depth, depth_gt, depth_mask, depth_gt_mask,
                            xy1_grid, Ts, cfg, b) for b in range(n_cores)]


_CACHED = {}


def _get_nc(cfg_key=(352, 1216, 32)):
    if cfg_key not in _CACHED:
        cfg = Cfg(*cfg_key)
        nc = bass.Bass()
        emit(nc, cfg)
        _CACHED[cfg_key] = (nc, cfg)
    return _CACHED[cfg_key]


def kernel(rgb, depth, depth_gt, depth_mask, depth_gt_mask, xy1_grid, Ts,
           **run_kwargs):
    from concourse.bass_utils import run_bass_kernel_spmd
    nc, cfg = _get_nc()
    maps = make_in_maps(rgb, depth, depth_gt, depth_mask, depth_gt_mask,
                        xy1_grid, Ts, cfg)
    res = run_bass_kernel_spmd(nc, maps, list(range(8)), **run_kwargs)
    total = np.float64(0.0)
    for r in res.results:
        total += np.float64(r["out_d"][:, 0].sum())
    n_gt = max(np.asarray(depth_gt_mask, np.float64).sum(), 1.0)
    loss = -total / n_gt
    kernel.last_results = res
    return np.float32(loss)
